# revision 4
# baseline (speedup 1.0000x reference)
"""MAPOCA fused kernel for 8 trn2 NeuronCores.

Strategy (validated against reference in numpy):
- The counterfactual Q-block attention collapses algebraically: slot m=0 is a
  full 768x768 attention over U-tokens; slots m>=1 reduce to 96x96 Gram blocks
  between W-tokens of agent pair {m-1, m}, with the multiplicities m / (16-m)
  folded into the exp as per-partition ln-biases.
- Everything is computed feature-major on-chip (weights pre-transposed by PE).
- SPMD over 8 cores: core c owns b-slice tokens [96c, 96c+96) of the m=0
  attention + outputs, and m-slots {2c+1, 2c+2} (core 7's second slot m=16 is
  computed on zero-padded data and discarded by the host).
- Per-core divergence happens ONLY through dynamic gpsimd DMAs (register
  offsets on compute-engine APs fault this hardware).
- Host precomputes: packed weights, packed biases (incl. mob' = mow@mib_v+mob,
  folding the V bias through the attention), ln-weight table, den/bcast masks.
"""
import os
import numpy as np

import concourse.bass as bass
import concourse.bacc as bacc_mod
import concourse.mybir as mybir
from concourse.tile import TileContext
from concourse.bass_utils import run_bass_kernel_spmd
from concourse.masks import make_identity

F32 = mybir.dt.float32
U32 = mybir.dt.uint32
AF = mybir.ActivationFunctionType
OP = mybir.AluOpType

B, N, D = 48, 16, 128
HID, A, H, hd = 128, 32, 4, 32
L = B * N            # 768
NC = 8
TL = L // NC         # 96 tokens per core (b-major)
SC = 1.0 / np.sqrt(hd)

# bias_pack column layout
BCOLS = {}
for i, nm in enumerate(
    ["ab1a", "ab1b", "ab2", "ab3", "eob", "eab", "rib", "mobp", "rob", "qb"]
    + [f"mib_{k}{h}" for k in "qk" for h in range(4)]):
    BCOLS[nm] = i
NBCOL = len(BCOLS)

# wpack slot layout (each slot [128,128], zero padded)
WSLOT = ["aw1a", "aw1b", "aw2k0", "aw2k1", "aw3", "eow", "eawo", "eawp",
         "riw", "miq", "mik", "miv", "mow", "row", "qw"]
NW = len(WSLOT)


def build_nc():
    nc = bacc_mod.Bacc(None, target_bir_lowering=False)
    d_obs = nc.declare_dram_parameter("obs", [B, N, D], F32, isOutput=False)
    d_wp = nc.declare_dram_parameter("wpack", [NW, 128, 128], F32, isOutput=False)
    d_bp = nc.declare_dram_parameter("bpack", [128, NBCOL], F32, isOutput=False)
    d_ln = nc.declare_dram_parameter("lnw", [96, 16], F32, isOutput=False)
    d_dm = nc.declare_dram_parameter("denmask", [128, 4, 128], F32, isOutput=False)
    d_bm = nc.declare_dram_parameter("bmask", [128, 128], F32, isOutput=False)
    d_off = nc.declare_dram_parameter("offs", [1, 4], U32, isOutput=False)
    o_pol = nc.declare_dram_parameter("o_pol", [TL, A], F32, isOutput=True)
    o_eo = nc.declare_dram_parameter("o_eo", [TL, D], F32, isOutput=True)
    o_qv = nc.declare_dram_parameter("o_qv", [1, 3 * TL], F32, isOutput=True)

    simsafe = os.environ.get("BASS_SIMSAFE") == "1"
    LRELU = AF.Relu if simsafe else AF.Lrelu
    GELU = AF.Relu if simsafe else AF.Gelu

    with TileContext(nc) as tc:
        with (
            tc.tile_pool(name="cst", bufs=1) as cst,
            tc.tile_pool(name="sb", bufs=1) as sb,
            tc.tile_pool(name="ps", bufs=1, space="PSUM") as pp,
        ):
            # ---------- constants in ----------
            ident = cst.tile([128, 128], F32)
            make_identity(nc, ident)
            obs_tok = cst.tile([128, 6, 128], F32)
            nc.sync.dma_start(out=obs_tok,
                              in_=d_obs.ap().rearrange("b n d -> (b n) d")
                              .rearrange("(n p) d -> p n d", p=128))
            wp = cst.tile([128, NW, 128], F32)
            nc.sync.dma_start(out=wp, in_=d_wp.ap().rearrange("t p f -> p t f"))
            bp = cst.tile([128, NBCOL], F32)
            nc.sync.dma_start(out=bp, in_=d_bp[:, :])
            lnw = cst.tile([96, 16], F32)
            nc.sync.dma_start(out=lnw, in_=d_ln[:, :])
            dmk = cst.tile([128, 4, 128], F32)
            nc.sync.dma_start(out=dmk, in_=d_dm[:, :, :])
            bmk = cst.tile([128, 128], F32)
            nc.sync.dma_start(out=bmk, in_=d_bm[:, :])
            offt = cst.tile([1, 4], U32)
            nc.sync.dma_start(out=offt, in_=d_off[:, :])

            def bcol(nm):
                return bp[:, BCOLS[nm]:BCOLS[nm] + 1]

            def bcol32(nm, p0):
                return bp[p0:p0 + 32, BCOLS[nm]:BCOLS[nm] + 1]

            # per-core offsets -> gpsimd registers (DMA-only usage!)
            r0 = nc.gpsimd.alloc_register("offb_r")
            nc.gpsimd.reg_load(r0, offt[0:1, 0:1])
            offb = nc.gpsimd.snap(r0, min_val=0, max_val=672)
            r1 = nc.gpsimd.alloc_register("offj_r")
            nc.gpsimd.reg_load(r1, offt[0:1, 1:2])
            offj = nc.gpsimd.snap(r1, min_val=0, max_val=672)
            r2 = nc.gpsimd.alloc_register("offl_r")
            nc.gpsimd.reg_load(r2, offt[0:1, 2:3])
            offl = nc.gpsimd.snap(r2, min_val=0, max_val=14)

            # ---------- weight transposes ----------
            wT = {}
            for i, nm in enumerate(WSLOT):
                tp = pp.tile([128, 128], F32, tag="ps")
                nc.tensor.transpose(tp, wp[:, i, :], ident)
                t = cst.tile([128, 128], F32, tag=f"wT_{nm}")
                nc.vector.tensor_copy(t, tp)
                wT[nm] = t

            # ---------- obs transpose (feature-major) ----------
            obsT = sb.tile([128, 768], F32)
            for i in range(6):
                tp = pp.tile([128, 128], F32, tag="ps")
                nc.tensor.transpose(tp, obs_tok[:, i, :], ident)
                nc.vector.tensor_copy(obsT[:, i * 128:(i + 1) * 128], tp)

            # j-major reorder (token (b,j) -> column j*48+b), zero padded to 816
            obsTj = sb.tile([128, 816], F32)
            nc.vector.memset(obsTj, 0.0)
            nc.vector.tensor_copy(
                obsTj[:, 0:768].rearrange("p (j b) -> p j b", j=16),
                obsT.rearrange("p (b j) -> p j b", j=16))

            # ---------- actor front (feature-major, 2 chunks of 384) ----------
            CH = (slice(0, 384), slice(384, 768))
            h1a = sb.tile([128, 768], F32)
            h1b = sb.tile([128, 768], F32)
            h2 = sb.tile([128, 768], F32)
            polT = sb.tile([32, 768], F32)

            # dummy gelu first: pin ACT to gelu_and_others for the whole front
            dg_in = cst.tile([1, 1], F32)
            nc.vector.memset(dg_in, 0.0)
            dg_out = cst.tile([1, 1], F32)
            nc.scalar.activation(dg_out, dg_in, GELU)

            for ci, c in enumerate(CH):
                for of, (wnm, bnm, dst) in enumerate(
                        [("aw1a", "ab1a", h1a), ("aw1b", "ab1b", h1b)]):
                    p = pp.tile([128, 384], F32, tag="ps")
                    nc.tensor.matmul(p, wT[wnm], obsT[:, c], start=True, stop=True)
                    nc.scalar.activation(dst[:, c], p, LRELU, bias=bcol(bnm), alpha=0.01)
            for ci, c in enumerate(CH):
                p = pp.tile([128, 384], F32, tag="ps")
                nc.tensor.matmul(p, wT["aw2k0"], h1a[:, c], start=True, stop=False)
                nc.tensor.matmul(p, wT["aw2k1"], h1b[:, c], start=False, stop=True)
                nc.scalar.activation(h2[:, c], p, LRELU, bias=bcol("ab2"), alpha=0.01)
            for ci, c in enumerate(CH):
                p = pp.tile([32, 384], F32, tag="ps32")
                nc.tensor.matmul(p, wT["aw3"][:, 0:32], h2[:, c], start=True, stop=True)
                nc.scalar.activation(polT[:, c], p, GELU, bias=bp[0:32, BCOLS["ab3"]:BCOLS["ab3"] + 1])

            polTj = sb.tile([32, 816], F32)
            nc.vector.memset(polTj, 0.0)
            nc.vector.tensor_copy(
                polTj[:, 0:768].rearrange("p (j b) -> p j b", j=16),
                polT.rearrange("p (b j) -> p j b", j=16))

            # ---------- state encoder (U track) ----------
            eoT = sb.tile([128, 768], F32)
            uT = sb.tile([128, 768], F32)
            for c in CH:
                p = pp.tile([128, 384], F32, tag="ps")
                nc.tensor.matmul(p, wT["eow"], obsT[:, c], start=True, stop=True)
                nc.vector.tensor_scalar_add(eoT[:, c], p, bcol("eob"))
            for c in CH:
                p = pp.tile([128, 384], F32, tag="ps")
                nc.tensor.matmul(p, wT["riw"], eoT[:, c], start=True, stop=True)
                nc.vector.tensor_scalar(uT[:, c], p, scalar1=bcol("rib"), scalar2=0.0,
                                        op0=OP.add, op1=OP.max)

            # dummy exp right after the gelus: preload exp set off critical path
            dex = cst.tile([1, 1], F32)
            nc.scalar.activation(dex, dg_out, AF.Exp)

            # ---------- dynamic per-core slices (gpsimd DMA only) ----------
            u_loc = sb.tile([128, 96], F32)
            nc.gpsimd.dma_start(out=u_loc, in_=uT[:, bass.ds(offb, 96)])
            obsj_loc = sb.tile([128, 144], F32)
            nc.gpsimd.dma_start(out=obsj_loc, in_=obsTj[:, bass.ds(offj, 144)])
            polj_loc = sb.tile([32, 144], F32)
            nc.gpsimd.dma_start(out=polj_loc, in_=polTj[:, bass.ds(offj, 144)])
            lnw_loc = sb.tile([96, 2], F32)
            nc.gpsimd.dma_start(out=lnw_loc, in_=lnw[:, bass.ds(offl, 2)])
            pol_loc = sb.tile([32, 96], F32)
            nc.gpsimd.dma_start(out=pol_loc, in_=polT[:, bass.ds(offb, 96)])
            eo_loc = sb.tile([128, 96], F32)
            nc.gpsimd.dma_start(out=eo_loc, in_=eoT[:, bass.ds(offb, 96)])

            # ---------- W track (local 144 j-major tokens) ----------
            eaj = sb.tile([128, 144], F32)
            p = pp.tile([128, 384], F32, tag="ps")
            nc.tensor.matmul(p[:, 0:144], wT["eawo"], obsj_loc, start=True, stop=False)
            nc.tensor.matmul(p[:, 0:144], wT["eawp"][0:32, :], polj_loc, start=False, stop=True)
            nc.vector.tensor_scalar_add(eaj, p[:, 0:144], bcol("eab"))
            wj = sb.tile([128, 144], F32)
            p = pp.tile([128, 384], F32, tag="ps")
            nc.tensor.matmul(p[:, 0:144], wT["riw"], eaj, start=True, stop=True)
            nc.vector.tensor_scalar(wj, p[:, 0:144], scalar1=bcol("rib"), scalar2=0.0,
                                    op0=OP.add, op1=OP.max)

            # ---------- projections ----------
            # KU per head [32, 768]
            KU = []
            for h in range(4):
                t = sb.tile([32, 768], F32, tag=f"KU{h}")
                for c in CH:
                    p = pp.tile([32, 384], F32, tag="ps32")
                    nc.tensor.matmul(p, wT["mik"][:, 32 * h:32 * h + 32], uT[:, c],
                                     start=True, stop=True)
                    nc.vector.tensor_scalar_add(t[:, c], p, bcol32(f"mib_k{h}", 0))
                KU.append(t)
            # VU token-major [128, 6, 128] (no bias: folded into mob')
            vu = sb.tile([128, 6, 128], F32)
            for i in range(6):
                p = pp.tile([128, 128], F32, tag="ps")
                nc.tensor.matmul(p, uT[:, i * 128:(i + 1) * 128], wT["miv"],
                                 start=True, stop=True)
                nc.vector.tensor_copy(vu[:, i, :], p)
            # QU local per head [32, 4, 96]
            qul = sb.tile([32, 4, 96], F32)
            for h in range(4):
                p = pp.tile([32, 96], F32, tag="ps32")
                nc.tensor.matmul(p, wT["miq"][:, 32 * h:32 * h + 32], u_loc,
                                 start=True, stop=True)
                nc.vector.tensor_scalar_add(qul[:, h, :], p, bcol32(f"mib_q{h}", 0))
            # KWj / QWj local per head [32, 144]
            kwj = sb.tile([32, 4, 144], F32)
            qwj = sb.tile([32, 4, 144], F32)
            for h in range(4):
                p = pp.tile([32, 144], F32, tag="ps32")
                nc.tensor.matmul(p, wT["mik"][:, 32 * h:32 * h + 32], wj, start=True, stop=True)
                nc.vector.tensor_scalar_add(kwj[:, h, :], p, bcol32(f"mib_k{h}", 0))
                p = pp.tile([32, 144], F32, tag="ps32")
                nc.tensor.matmul(p, wT["miq"][:, 32 * h:32 * h + 32], wj, start=True, stop=True)
                nc.vector.tensor_scalar_add(qwj[:, h, :], p, bcol32(f"mib_q{h}", 0))
            # VW token-major per m-slot [96, 128]
            vwm = []
            for mi in range(2):
                t = sb.tile([96, 128], F32, tag=f"vwm{mi}")
                p = pp.tile([96, 128], F32, tag="ps")
                nc.tensor.matmul(p, wj[:, 48 * mi:48 * mi + 96], wT["miv"],
                                 start=True, stop=True)
                nc.vector.tensor_copy(t, p)
                vwm.append(t)

            # ---------- m=0 attention: scores + exp ----------
            # slots (h, pt) flat, packed 5 per psum bank
            ES = []  # list of (sbuf_tile, nslots); slot s of pack -> (h,pt)
            slots = [(h, pt) for h in range(4) for pt in range(6)]
            packs = [slots[i:i + 5] for i in range(0, 24, 5)]
            for pk in packs:
                n = len(pk)
                sp = pp.tile([128, 5, 96], F32, tag="st")
                for s, (h, pt) in enumerate(pk):
                    nc.tensor.matmul(sp[:, s, :], KU[h][:, 128 * pt:128 * (pt + 1)],
                                     qul[:, h, :], start=True, stop=True)
                es = sb.tile([128, 5, 96], F32, tag=f"es{len(ES)}")
                nc.scalar.activation(es[:, 0:n, :], sp[:, 0:n, :], AF.Exp, scale=SC)
                ES.append((es, n))

            def es_slot(h, pt):
                idx = h * 6 + pt
                return ES[idx // 5][0][:, idx % 5, :]

            # ---------- m=0 AV + den ----------
            ao = pp.tile([128, 96], F32, tag="ao")
            for pt in range(6):
                for h in range(4):
                    nc.tensor.matmul(ao[32 * h:32 * h + 32, :], vu[:, pt, 32 * h:32 * h + 32],
                                     es_slot(h, pt), start=(pt == 0), stop=(pt == 5),
                                     tile_position=(0, 32 * h))
            den = pp.tile([128, 96], F32, tag="den")
            first = True
            for pt in range(6):
                for h in range(4):
                    nc.tensor.matmul(den, dmk[:, h, :], es_slot(h, pt),
                                     start=first, stop=(pt == 5 and h == 3))
                    first = False

            aocat = sb.tile([128, 3, 96], F32)

            def divide(ao_ps, den_ps, out_ap):
                rden = sb.tile([128, 96], F32, tag="rden")
                nc.vector.memset(rden, 0.0)
                for h in range(4):
                    nc.vector.reciprocal(rden[32 * h:32 * h + 1, :],
                                         den_ps[32 * h:32 * h + 1, :])
                rb = pp.tile([128, 96], F32, tag="rb")
                nc.tensor.matmul(rb, bmk, rden, start=True, stop=True)
                rbs = sb.tile([128, 96], F32, tag="rbs")
                nc.vector.tensor_copy(rbs, rb)
                nc.vector.tensor_mul(out_ap, ao_ps, rbs)

            divide(ao, den, aocat[:, 0, :])

            # ---------- m>=1 slots: gram + exp + AV ----------
            for mi in range(2):
                gp = pp.tile([96, 4, 96], F32, tag="st")
                for h in range(4):
                    nc.tensor.matmul(gp[:, h, :], kwj[:, h, 48 * mi:48 * mi + 96],
                                     qwj[:, h, 48 * mi:48 * mi + 96], start=True, stop=True)
                eg = sb.tile([96, 4, 96], F32, tag="eg")
                nc.scalar.activation(eg, gp, AF.Exp, scale=SC,
                                     bias=lnw_loc[:, mi:mi + 1])
                aow = pp.tile([128, 96], F32, tag="ao")
                for h in range(4):
                    nc.tensor.matmul(aow[32 * h:32 * h + 32, :], vwm[mi][:, 32 * h:32 * h + 32],
                                     eg[:, h, :], start=True, stop=True,
                                     tile_position=(0, 32 * h))
                den2 = pp.tile([128, 96], F32, tag="den")
                for h in range(4):
                    nc.tensor.matmul(den2, dmk[0:96, h, :], eg[:, h, :],
                                     start=(h == 0), stop=(h == 3))
                divide(aow, den2, aocat[:, 1 + mi, :])

            # ---------- epilogue (batched over the 3 segments) ----------
            ap_ps = pp.tile([128, 288], F32, tag="ps")
            nc.tensor.matmul(ap_ps, wT["mow"], aocat[:, :, :], start=True, stop=True)
            aplus = sb.tile([128, 3, 96], F32)
            nc.vector.tensor_scalar_add(aplus, ap_ps, bcol("mobp"))
            nc.vector.tensor_add(aplus[:, 0, :], aplus[:, 0, :], u_loc)
            # x1 for slot mi is wj columns [48*mi, 48*mi+96) - overlapping slices
            nc.vector.tensor_add(aplus[:, 1, :], aplus[:, 1, :], wj[:, 0:96])
            nc.vector.tensor_add(aplus[:, 2, :], aplus[:, 2, :], wj[:, 48:144])
            c_ps = pp.tile([128, 288], F32, tag="ps")
            nc.tensor.matmul(c_ps, wT["row"], aplus, start=True, stop=True)
            csb = sb.tile([128, 3, 96], F32)
            nc.vector.tensor_scalar(csb, c_ps, scalar1=bcol("rob"), scalar2=0.0,
                                    op0=OP.add, op1=OP.max)
            qv_ps = pp.tile([1, 288], F32, tag="qv")
            nc.tensor.matmul(qv_ps, wT["qw"][:, 0:1], csb, start=True, stop=True)
            qvs = sb.tile([1, 288], F32)
            nc.vector.tensor_scalar_add(qvs, qv_ps, bp[0:1, BCOLS["qb"]:BCOLS["qb"] + 1])
            nc.sync.dma_start(out=o_qv[:, :], in_=qvs)

            # ---------- policy / eo outputs (token-major local slices) ----------
            tp = pp.tile([128, 128], F32, tag="ps")
            nc.tensor.transpose(tp[0:96, 0:32], pol_loc, ident[0:32, 0:32])
            pol_out = sb.tile([96, 32], F32)
            nc.vector.tensor_copy(pol_out, tp[0:96, 0:32])
            nc.sync.dma_start(out=o_pol[:, :], in_=pol_out)
            tp2 = pp.tile([128, 128], F32, tag="ps")
            nc.tensor.transpose(tp2[0:96, :], eo_loc, ident)
            eo_out = sb.tile([96, 128], F32)
            nc.vector.tensor_copy(eo_out, tp2[0:96, :])
            nc.sync.dma_start(out=o_eo[:, :], in_=eo_out)

    nc.compile()
    return nc


def pack_inputs(inp):
    """Build the per-core input maps from the raw reference inputs."""
    wpack = np.zeros((NW, 128, 128), np.float32)
    def put(nm, mat):
        s = WSLOT.index(nm)
        wpack[s, :mat.shape[0], :mat.shape[1]] = mat
    put("aw1a", inp["aw1"][0:128]); put("aw1b", inp["aw1"][128:256])
    put("aw2k0", inp["aw2"][:, 0:128]); put("aw2k1", inp["aw2"][:, 128:256])
    put("aw3", inp["aw3"]); put("eow", inp["eow"])
    put("eawo", inp["eaw"][:, 0:128]); put("eawp", inp["eaw"][:, 128:160])
    put("riw", inp["riw"])
    put("miq", inp["miw"][0:128]); put("mik", inp["miw"][128:256]); put("miv", inp["miw"][256:384])
    put("mow", inp["mow"]); put("row", inp["row"]); put("qw", inp["qw"])

    bpack = np.zeros((128, NBCOL), np.float32)
    def putb(nm, vec):
        bpack[:len(vec), BCOLS[nm]] = vec
    putb("ab1a", inp["ab1"][0:128]); putb("ab1b", inp["ab1"][128:256])
    putb("ab2", inp["ab2"]); putb("ab3", inp["ab3"])
    putb("eob", inp["eob"]); putb("eab", inp["eab"]); putb("rib", inp["rib"])
    putb("mobp", inp["mow"] @ inp["mib"][256:384] + inp["mob"])
    putb("rob", inp["rob"]); putb("qb", inp["qb"])
    for h in range(4):
        putb(f"mib_q{h}", inp["mib"][32 * h:32 * h + 32])
        putb(f"mib_k{h}", inp["mib"][128 + 32 * h:128 + 32 * h + 32])

    lnw = np.zeros((96, 16), np.float32)
    for m in range(1, 16):
        lnw[0:48, m - 1] = np.log(16.0 - m)   # rows j' = m-1
        lnw[48:96, m - 1] = np.log(float(m))  # rows j' = m
    dm = np.zeros((128, 4, 128), np.float32)
    for h in range(4):
        dm[:, h, 32 * h] = 1.0
    bm = np.zeros((128, 128), np.float32)
    for h in range(4):
        bm[32 * h, 32 * h:32 * (h + 1)] = 1.0

    obs = np.ascontiguousarray(np.asarray(inp["obs"], np.float32))
    maps = []
    for c in range(NC):
        offs = np.array([[c * 96, c * 96, min(2 * c, 14), 0]], np.uint32)
        maps.append(dict(obs=obs, wpack=wpack, bpack=bpack, lnw=lnw,
                         denmask=dm, bmask=bm, offs=offs))
    return maps


_NC_CACHE = None


def kernel(**inputs):
    global _NC_CACHE
    if _NC_CACHE is None:
        _NC_CACHE = build_nc()
    nc = _NC_CACHE
    maps = pack_inputs(inputs)
    res = run_bass_kernel_spmd(nc, maps, list(range(NC)))
    pol = np.zeros((L, A), np.float32)
    eo = np.zeros((L, D), np.float32)
    qv0 = np.zeros(L, np.float32)
    qvw = np.zeros((16, 2, B), np.float32)   # [m-1, z, b] for m=1..15 (+pad)
    for c in range(NC):
        r = res.results[c]
        pol[c * 96:(c + 1) * 96] = r["o_pol"]
        eo[c * 96:(c + 1) * 96] = r["o_eo"]
        q = r["o_qv"][0]
        qv0[c * 96:(c + 1) * 96] = q[0:96]
        for mi in range(2):
            m = 2 * c + 1 + mi
            if m <= 15:
                seg = q[96 + 96 * mi: 96 + 96 * (mi + 1)]
                qvw[m - 1, 0] = seg[0:48]    # z=0: jq=m-1
                qvw[m - 1, 1] = seg[48:96]   # z=1: jq=m
    qv = qv0.reshape(B, N).copy()
    for i in range(N):
        for m in range(1, 16):
            z = 1 if i < m else 0
            qv[:, i] += qvw[m - 1, z]
    return (pol.reshape(B, N * A),
            qv.reshape(B, N, 1).astype(np.float32),
            eo.reshape(B, N, D))
